# revision 27
# baseline (speedup 1.0000x reference)
"""Trainium2 Bass kernel for nn_PizzaBurningEffect.

Reference computation (per batch b):
    ew[h,w]   : fixed edge-weight grid (input-independent)
    spots     = max_s exp(-((x_w-sx)^2+(y_h-sy)^2)/(2 r_s^2)) * sint_s
    bm        = clip(max(ew, spots) * burn_b, 0, 1)
    out[c]    = img[c] * (1 - kappa_c * burn_b * max(ew, spots)),
                kappa_c = 1 - dark_c
(The clips are no-ops: every operand is in [0,1) and bm <= 0.8.)

Device strategy (p-norm max on the tensor engine):
    max_s g_s ~= (sum_s g_s^32)^(1/32)
The 32nd powers are separable: g_s^32 = gyp_s(h) * gxp_s(w), with the tiny
1-D tables gyp/gxp computed on the host (scaled by sqrt(LAM)=3.16e18 each so
fp32/bf16 dynamic range covers g in [0.017, 1]; smaller factors flush to 0).
Per 128-row chunk the sum over s is ONE 8x128 x 8x512 bf16 matmul into PSUM.
The 1/32 root is a single ACT Exp on the *bitcast-int32* view of the PSUM
sum: the int32 pattern of an fp32 is linear in log2 (max bit-log error
0.086 log2 / 32 -> <0.1% after centring), so Exp(scale*I + bias) with
scale = ln2/(32*2^23) computes (S/LAM)^(1/32) over the full fp32 range.
(ACT's Ln table clamps below 3e-20 and is garbage above 2.5e19, so a real
Ln+Exp root cannot cover the 70-decade range of the 32nd powers.)  A small
deflation delta folded into the Exp bias centres the p-norm overshoot.
DVE then does: max with ew (fp16), three tensor_scalar ops for
F_c = 1 - kappa_c*burn*bm (4x packed), and one wide 16-bit multiply.

img/out travel as bf16 (fp16's subnormal range breaks the rel-err metric at
tiny image values), host-packed into chunk-contiguous [b,k,p,c*w] layout so
every img/out DMA is large contiguous 3KB-per-partition blocks.  TRN2 has
two HW-DGE rings (one fed by the SP engine, one by ACT): img loads go on
the SP ring, table/ew loads and output stores on the ACT ring, so load and
store traffic move in parallel.

Sharding: pure data parallel, 4 batches per core on 8 cores.
"""

import numpy as np
import ml_dtypes

import concourse.bacc as bacc
import concourse.bass as bass
from concourse import mybir
from concourse.tile import TileContext
from concourse.bass_utils import run_bass_kernel_spmd

B, C, H, W, S = 32, 3, 512, 512, 8
NCORES = 8
BL = B // NCORES          # batches per core
P = 128                   # partitions
K = H // P                # row chunks per image
SR = S                   # matmul contraction rows
DT = mybir.dt.float32
DTH = mybir.dt.float16    # mask chain
DTB = mybir.dt.bfloat16   # img/out + power tables
NPB = ml_dtypes.bfloat16

BURN_MIN, BURN_MAX = 0.2, 0.8
DARK = np.array([0.7, 0.4, 0.3], dtype=np.float64)

PNORM = 32.0
LAM = 1e37                # sum scale; sqrt(LAM) per 1-D factor
DELTA = 0.0065            # deflation centring the p-norm overshoot
SIGMA = -0.0430           # bit-log centring constant
EXP_SCALE = float(np.log(2.0) / (PNORM * 2.0 ** 23))
EXP_BIAS = float(np.log(2.0) * (-127.0 - SIGMA) / PNORM
                 - np.log(LAM) / PNORM + np.log1p(-DELTA))


def _build_program():
    nc = bacc.Bacc("TRN2", target_bir_lowering=False, debug=False,
                   num_devices=NCORES)

    img = nc.dram_tensor("img", [BL, K, P, C * W], DTB, kind="ExternalInput")
    gxp = nc.dram_tensor("gxp", [SR, BL, W], DTB, kind="ExternalInput")
    gyp = nc.dram_tensor("gyp", [SR, BL, K, P], DTB, kind="ExternalInput")
    ew = nc.dram_tensor("ew", [P, K, W], DTH, kind="ExternalInput")
    s1 = nc.dram_tensor("s1", [P, BL, C], DT, kind="ExternalInput")
    out = nc.dram_tensor("out", [BL, K, P, C * W], DTB, kind="ExternalOutput")

    mx = mybir.AluOpType.max
    mult = mybir.AluOpType.mult
    add = mybir.AluOpType.add

    with TileContext(nc) as tc:
        with (
            tc.tile_pool(name="singles", bufs=1) as singles,
            tc.tile_pool(name="imgp", bufs=6) as imgp,
            tc.tile_pool(name="outp", bufs=4) as outp,
            tc.tile_pool(name="spp", bufs=4) as spp,
            tc.tile_pool(name="bmp", bufs=4) as bmp,
            tc.tile_pool(name="fp", bufs=4) as fpool,
            tc.psum_pool(name="qp", bufs=3) as qpool,
        ):
            gyp_t = singles.tile([SR, BL, K, P], DTB)
            nc.sync.dma_start(out=gyp_t[:], in_=gyp[:])
            gxp_t = singles.tile([SR, BL, W], DTB)
            nc.sync.dma_start(out=gxp_t[:], in_=gxp[:])
            s1_t = singles.tile([P, BL, C], DT)
            nc.sync.dma_start(out=s1_t[:], in_=s1[:])
            ew_t = singles.tile([P, K, W], DTH)
            for k in range(K):
                nc.scalar.dma_start(out=ew_t[:, k, :], in_=ew[:, k, :])

            bias_t = singles.tile([P, 1], DT)
            nc.vector.memset(bias_t[:], EXP_BIAS)

            # Warm the exp ACT table set during the initial DMAs.
            warm = singles.tile([P, 2], DT)
            nc.vector.memset(warm[:, 0:1], 1.0)
            nc.scalar.activation(out=warm[:, 1:2], in_=warm[:, 0:1],
                                 func=mybir.ActivationFunctionType.Exp)

            # Software-pipelined by one double-chunk: F2 (ACT) and the
            # blend+store of iteration i are emitted during iteration i+1,
            # so ACT's in-order stream never stalls waiting on DVE's bm.
            prev = None
            for b in range(BL):
                for j in range(K // 2):
                    img_t = imgp.tile([P, 2, C * W], DTB)
                    if b == 0 and j == 0:
                        # split the very first load so chunk 0 lands sooner
                        for kk in range(2):
                            nc.sync.dma_start(
                                out=img_t[:, kk, :],
                                in_=img[b, 2 * j + kk])
                    else:
                        nc.sync.dma_start(
                            out=img_t[:],
                            in_=img[b, 2 * j:2 * j + 2].rearrange(
                                "k p f -> p k f"))

                    # q = sum_s gyp_s(p) * gxp_s(w)  (PE outer products)
                    q = qpool.tile([P, 2, W], DT)
                    for kk in range(2):
                        nc.tensor.matmul(
                            q[:, kk, :], gyp_t[:, b, 2 * j + kk, :],
                            gxp_t[:, b, :])

                    # spots = (q/LAM)^(1/32)*(1-DELTA): one Exp on the
                    # bitcast-int32 PSUM view (bit-trick logarithm)
                    sp = spp.tile([P, 2, W], DTH)
                    nc.scalar.activation(
                        out=sp[:], in_=q[:].bitcast(mybir.dt.int32),
                        func=mybir.ActivationFunctionType.Exp,
                        bias=bias_t[:], scale=EXP_SCALE)

                    if prev is not None:
                        pb, pj, pimg, pf, pbm = prev
                        nc.scalar.activation(
                            out=pf[:, :, 2, :], in_=pbm[:],
                            func=mybir.ActivationFunctionType.Identity,
                            bias=1.0, scale=s1_t[:, pb, 2:3])

                    # bm = max(spots, ew)
                    bm = bmp.tile([P, 2, W], DTH)
                    nc.vector.tensor_tensor(
                        out=bm[:], in0=sp[:],
                        in1=ew_t[:, 2 * j:2 * j + 2, :], op=mx)

                    # F_c = 1 - kappa_c*burn_b*bm (c=0,1 on DVE, 4x packed)
                    f_t = fpool.tile([P, 2, C, W], DTH)
                    for c in range(2):
                        nc.vector.tensor_scalar(
                            out=f_t[:, :, c, :], in0=bm[:],
                            scalar1=s1_t[:, b, c:c + 1], scalar2=1.0,
                            op0=mult, op1=add)

                    if prev is not None:
                        pb, pj, pimg, pf, pbm = prev
                        out_t = outp.tile([P, 2, C * W], DTB)
                        nc.vector.tensor_tensor(
                            out=out_t[:], in0=pimg[:],
                            in1=pf[:].rearrange("p k c w -> p k (c w)"),
                            op=mult)
                        # alternate stores across both HW-DGE rings
                        eng = nc.sync if (2 * pb + pj) % 2 else nc.scalar
                        eng.dma_start(
                            out=out[pb, 2 * pj:2 * pj + 2].rearrange(
                                "k p f -> p k f"),
                            in_=out_t[:])
                    prev = (b, j, img_t, f_t, bm)

            # drain the last double-chunk
            pb, pj, pimg, pf, pbm = prev
            nc.scalar.activation(
                out=pf[:, :, 2, :], in_=pbm[:],
                func=mybir.ActivationFunctionType.Identity,
                bias=1.0, scale=s1_t[:, pb, 2:3])
            out_t = outp.tile([P, 2, C * W], DTB)
            nc.vector.tensor_tensor(
                out=out_t[:], in0=pimg[:],
                in1=pf[:].rearrange("p k c w -> p k (c w)"), op=mult)
            nc.sync.dma_start(
                out=out[pb, 2 * pj:2 * pj + 2].rearrange("k p f -> p k f"),
                in_=out_t[:])

    nc.compile()
    return nc


_NC = None


def _get_nc():
    global _NC
    if _NC is None:
        _NC = _build_program()
    return _NC


def _host_tables(u_xy, u_radius, u_intensity, u_burn):
    """1-D 32nd-power tables (float64 host math, bf16 on device)."""
    u_xy = np.asarray(u_xy, np.float64)
    u_radius = np.asarray(u_radius, np.float64)
    u_intensity = np.asarray(u_intensity, np.float64)
    u_burn = np.asarray(u_burn, np.float64)

    y = np.linspace(-1.0, 1.0, H)
    x = np.linspace(-1.0, 1.0, W)

    spot_xy = 2.0 * u_xy - 1.0
    sx = spot_xy[..., 0]                   # [B,S]
    sy = spot_xy[..., 1]
    radius = 0.05 + 0.15 * u_radius
    sint = 0.5 + 0.5 * u_intensity
    inv2r2 = 1.0 / (2.0 * radius ** 2)
    burn = BURN_MIN + (BURN_MAX - BURN_MIN) * u_burn   # [B]

    lamh_log = 0.5 * np.log(LAM)
    # log of (sint*gx)^32 * sqrt(LAM) and gy^32 * sqrt(LAM)
    tx = PNORM * (-((x[None, None, :] - sx[..., None]) ** 2)
                  * inv2r2[..., None] + np.log(sint)[..., None]) + lamh_log
    ty = PNORM * (-((y[None, None, :] - sy[..., None]) ** 2)
                  * inv2r2[..., None]) + lamh_log
    gxp = np.where(tx > -87.0, np.exp(tx), 0.0)        # [B,S,W]
    gyp = np.where(ty > -87.0, np.exp(ty), 0.0)        # [B,S,H]

    # device layouts
    gxp_lay = np.ascontiguousarray(
        gxp.transpose(1, 0, 2)).astype(NPB)            # [SR,B,W]
    gyp_lay = np.ascontiguousarray(
        gyp.reshape(B, SR, K, P).transpose(1, 0, 2, 3)).astype(NPB)

    kappa = 1.0 - DARK                                 # [C]
    s1 = -(burn[:, None] * kappa[None, :])             # [B,C]
    s1_lay = np.ascontiguousarray(np.broadcast_to(
        s1.astype(np.float32), (P, B, C)))
    return gxp_lay, gyp_lay, s1_lay


def _edge_weight():
    y = np.linspace(-1.0, 1.0, H)
    x = np.linspace(-1.0, 1.0, W)
    yc, xc = np.meshgrid(y, x, indexing="ij")
    dist = np.sqrt(xc ** 2 + yc ** 2)
    ew = np.exp(2.0 * (dist - 0.7))
    ew = (ew - ew.min()) / (ew.max() - ew.min() + 1e-6)
    # ew_lay[p, k, w] = ew[k*P+p, w]
    return np.ascontiguousarray(
        ew.reshape(K, P, W).transpose(1, 0, 2).astype(np.float16))


_EW = None


def kernel(img, u_xy, u_radius, u_intensity, u_burn, _run_kwargs=None):
    global _EW
    img = np.asarray(img, np.float32)
    # pack to [B, K, P, C*W] bf16: chunk-contiguous DMA blocks
    img_dev = np.ascontiguousarray(
        img.reshape(B, C, K, P, W).transpose(0, 2, 3, 1, 4)
    ).astype(NPB).reshape(B, K, P, C * W)

    gxp_lay, gyp_lay, s1_lay = _host_tables(
        u_xy, u_radius, u_intensity, u_burn)
    if _EW is None:
        _EW = _edge_weight()

    nc = _get_nc()
    core_ids = list(range(NCORES))
    in_maps = []
    for i in core_ids:
        lo, hi = i * BL, (i + 1) * BL
        in_maps.append({
            "img": img_dev[lo:hi],
            "gxp": np.ascontiguousarray(gxp_lay[:, lo:hi]),
            "gyp": np.ascontiguousarray(gyp_lay[:, lo:hi]),
            "ew": _EW,
            "s1": np.ascontiguousarray(s1_lay[:, lo:hi]),
        })
    res = run_bass_kernel_spmd(nc, in_maps, core_ids, **(_run_kwargs or {}))
    out_dev = np.concatenate(
        [np.asarray(res.results[i]["out"]) for i in core_ids], axis=0)
    out = np.ascontiguousarray(
        out_dev.reshape(B, K, P, C, W).transpose(0, 3, 1, 2, 4)
    ).astype(np.float32).reshape(B, C, H, W)
    if _run_kwargs:
        kernel._last_results = res
    return out


# revision 28
# speedup vs baseline: 1.1907x; 1.1907x over previous
"""Trainium2 Bass kernel for nn_PizzaBurningEffect.

Reference computation (per batch b):
    ew[h,w]   : fixed edge-weight grid (input-independent)
    spots     = max_s exp(-((x_w-sx)^2+(y_h-sy)^2)/(2 r_s^2)) * sint_s
    bm        = clip(max(ew, spots) * burn_b, 0, 1)
    out[c]    = img[c] * (1 - kappa_c * burn_b * max(ew, spots)),
                kappa_c = 1 - dark_c
(The clips are no-ops: every operand is in [0,1) and bm <= 0.8.)

Device strategy (p-norm max on the tensor engine):
    max_s g_s ~= (sum_s g_s^32)^(1/32)
The 32nd powers are separable: g_s^32 = gyp_s(h) * gxp_s(w), with the tiny
1-D tables gyp/gxp computed on the host (scaled by sqrt(LAM)=3.16e18 each so
fp32/bf16 dynamic range covers g in [0.017, 1]; smaller factors flush to 0).
Per 128-row chunk the sum over s is ONE 8x128 x 8x512 bf16 matmul into PSUM.
The 1/32 root is a single ACT Exp on the *bitcast-int32* view of the PSUM
sum: the int32 pattern of an fp32 is linear in log2 (max bit-log error
0.086 log2 / 32 -> <0.1% after centring), so Exp(scale*I + bias) with
scale = ln2/(32*2^23) computes (S/LAM)^(1/32) over the full fp32 range.
(ACT's Ln table clamps below 3e-20 and is garbage above 2.5e19, so a real
Ln+Exp root cannot cover the 70-decade range of the 32nd powers.)  A small
deflation delta folded into the Exp bias centres the p-norm overshoot.
DVE then does: max with ew (fp16), three tensor_scalar ops for
F_c = 1 - kappa_c*burn*bm (4x packed), and one wide 16-bit multiply.

img/out travel as bf16 (fp16's subnormal range breaks the rel-err metric at
tiny image values), host-packed into chunk-contiguous [b,k,p,c*w] layout so
every img/out DMA is large contiguous 3KB-per-partition blocks.  TRN2 has
two HW-DGE rings (one fed by the SP engine, one by ACT): img loads go on
the SP ring, table/ew loads and output stores on the ACT ring, so load and
store traffic move in parallel.

Sharding: pure data parallel, 4 batches per core on 8 cores.
"""

import numpy as np
import ml_dtypes

import concourse.bacc as bacc
import concourse.bass as bass
from concourse import mybir
from concourse.tile import TileContext
from concourse.bass_utils import run_bass_kernel_spmd

B, C, H, W, S = 32, 3, 512, 512, 8
NCORES = 8
BL = B // NCORES          # batches per core
P = 128                   # partitions
K = H // P                # row chunks per image
SR = S                   # matmul contraction rows
DT = mybir.dt.float32
DTH = mybir.dt.float16    # mask chain
DTB = mybir.dt.bfloat16   # img/out + power tables
NPB = ml_dtypes.bfloat16

BURN_MIN, BURN_MAX = 0.2, 0.8
DARK = np.array([0.7, 0.4, 0.3], dtype=np.float64)

PNORM = 32.0
LAM = 1e37                # sum scale; sqrt(LAM) per 1-D factor
DELTA = 0.0065            # deflation centring the p-norm overshoot
SIGMA = -0.0430           # bit-log centring constant
EXP_SCALE = float(np.log(2.0) / (PNORM * 2.0 ** 23))
EXP_BIAS = float(np.log(2.0) * (-127.0 - SIGMA) / PNORM
                 - np.log(LAM) / PNORM + np.log1p(-DELTA))


def _build_program():
    nc = bacc.Bacc("TRN2", target_bir_lowering=False, debug=False,
                   num_devices=NCORES)

    img = nc.dram_tensor("img", [BL, K, P, C * W], DTB, kind="ExternalInput")
    gxp = nc.dram_tensor("gxp", [SR, BL, W], DTB, kind="ExternalInput")
    gyp = nc.dram_tensor("gyp", [SR, BL, K, P], DTB, kind="ExternalInput")
    ew = nc.dram_tensor("ew", [P, K, W], DTH, kind="ExternalInput")
    s1 = nc.dram_tensor("s1", [P, BL, C], DT, kind="ExternalInput")
    out = nc.dram_tensor("out", [BL, K, P, C * W], DTB, kind="ExternalOutput")

    mx = mybir.AluOpType.max
    mult = mybir.AluOpType.mult
    add = mybir.AluOpType.add

    with TileContext(nc) as tc:
        with (
            tc.tile_pool(name="singles", bufs=1) as singles,
            tc.tile_pool(name="imgp", bufs=6) as imgp,
            tc.tile_pool(name="outp", bufs=4) as outp,
            tc.tile_pool(name="spp", bufs=4) as spp,
            tc.tile_pool(name="bmp", bufs=4) as bmp,
            tc.tile_pool(name="fp", bufs=4) as fpool,
            tc.psum_pool(name="qp", bufs=3) as qpool,
        ):
            gyp_t = singles.tile([SR, BL, K, P], DTB)
            nc.sync.dma_start(out=gyp_t[:], in_=gyp[:])
            gxp_t = singles.tile([SR, BL, W], DTB)
            nc.sync.dma_start(out=gxp_t[:], in_=gxp[:])
            s1_t = singles.tile([P, BL, C], DT)
            nc.sync.dma_start(out=s1_t[:], in_=s1[:])
            ew_t = singles.tile([P, K, W], DTH)
            nc.scalar.dma_start(out=ew_t[:], in_=ew[:])

            bias_t = singles.tile([P, 1], DT)
            nc.vector.memset(bias_t[:], EXP_BIAS)

            # Warm the exp ACT table set during the initial DMAs.
            warm = singles.tile([P, 2], DT)
            nc.vector.memset(warm[:, 0:1], 1.0)
            nc.scalar.activation(out=warm[:, 1:2], in_=warm[:, 0:1],
                                 func=mybir.ActivationFunctionType.Exp)

            # Software-pipelined by one double-chunk: F2 (ACT) and the
            # blend+store of iteration i are emitted during iteration i+1,
            # so ACT's in-order stream never stalls waiting on DVE's bm.
            prev = None
            for b in range(BL):
                for j in range(K // 2):
                    img_t = imgp.tile([P, 2, C * W], DTB)
                    if b == 0 and j == 0:
                        # split the very first load so chunk 0 lands sooner
                        for kk in range(2):
                            nc.sync.dma_start(
                                out=img_t[:, kk, :],
                                in_=img[b, 2 * j + kk])
                    else:
                        nc.sync.dma_start(
                            out=img_t[:],
                            in_=img[b, 2 * j:2 * j + 2].rearrange(
                                "k p f -> p k f"))

                    # q = sum_s gyp_s(p) * gxp_s(w)  (PE outer products)
                    q = qpool.tile([P, 2, W], DT)
                    for kk in range(2):
                        nc.tensor.matmul(
                            q[:, kk, :], gyp_t[:, b, 2 * j + kk, :],
                            gxp_t[:, b, :])

                    # spots = (q/LAM)^(1/32)*(1-DELTA): one Exp on the
                    # bitcast-int32 PSUM view (bit-trick logarithm)
                    sp = spp.tile([P, 2, W], DTH)
                    nc.scalar.activation(
                        out=sp[:], in_=q[:].bitcast(mybir.dt.int32),
                        func=mybir.ActivationFunctionType.Exp,
                        bias=bias_t[:], scale=EXP_SCALE)

                    if prev is not None:
                        pb, pj, pimg, pf, pbm = prev
                        nc.scalar.activation(
                            out=pf[:, :, 2, :], in_=pbm[:],
                            func=mybir.ActivationFunctionType.Identity,
                            bias=1.0, scale=s1_t[:, pb, 2:3])

                    # bm = max(spots, ew)
                    bm = bmp.tile([P, 2, W], DTH)
                    nc.vector.tensor_tensor(
                        out=bm[:], in0=sp[:],
                        in1=ew_t[:, 2 * j:2 * j + 2, :], op=mx)

                    # F_c = 1 - kappa_c*burn_b*bm (c=0,1 on DVE, 4x packed)
                    f_t = fpool.tile([P, 2, C, W], DTH)
                    for c in range(2):
                        nc.vector.tensor_scalar(
                            out=f_t[:, :, c, :], in0=bm[:],
                            scalar1=s1_t[:, b, c:c + 1], scalar2=1.0,
                            op0=mult, op1=add)

                    if prev is not None:
                        pb, pj, pimg, pf, pbm = prev
                        out_t = outp.tile([P, 2, C * W], DTB)
                        nc.vector.tensor_tensor(
                            out=out_t[:], in0=pimg[:],
                            in1=pf[:].rearrange("p k c w -> p k (c w)"),
                            op=mult)
                        # alternate stores across both HW-DGE rings
                        eng = nc.sync if (2 * pb + pj) % 2 else nc.scalar
                        eng.dma_start(
                            out=out[pb, 2 * pj:2 * pj + 2].rearrange(
                                "k p f -> p k f"),
                            in_=out_t[:])
                    prev = (b, j, img_t, f_t, bm)

            # drain the last double-chunk; its F2 runs on DVE so the
            # tail never round-trips through ACT
            pb, pj, pimg, pf, pbm = prev
            nc.vector.tensor_scalar(
                out=pf[:, :, 2, :], in0=pbm[:],
                scalar1=s1_t[:, pb, 2:3], scalar2=1.0,
                op0=mult, op1=add)
            out_t = outp.tile([P, 2, C * W], DTB)
            nc.vector.tensor_tensor(
                out=out_t[:], in0=pimg[:],
                in1=pf[:].rearrange("p k c w -> p k (c w)"), op=mult)
            nc.sync.dma_start(
                out=out[pb, 2 * pj:2 * pj + 2].rearrange("k p f -> p k f"),
                in_=out_t[:])

    nc.compile()
    return nc


_NC = None


def _get_nc():
    global _NC
    if _NC is None:
        _NC = _build_program()
    return _NC


def _host_tables(u_xy, u_radius, u_intensity, u_burn):
    """1-D 32nd-power tables (float64 host math, bf16 on device)."""
    u_xy = np.asarray(u_xy, np.float64)
    u_radius = np.asarray(u_radius, np.float64)
    u_intensity = np.asarray(u_intensity, np.float64)
    u_burn = np.asarray(u_burn, np.float64)

    y = np.linspace(-1.0, 1.0, H)
    x = np.linspace(-1.0, 1.0, W)

    spot_xy = 2.0 * u_xy - 1.0
    sx = spot_xy[..., 0]                   # [B,S]
    sy = spot_xy[..., 1]
    radius = 0.05 + 0.15 * u_radius
    sint = 0.5 + 0.5 * u_intensity
    inv2r2 = 1.0 / (2.0 * radius ** 2)
    burn = BURN_MIN + (BURN_MAX - BURN_MIN) * u_burn   # [B]

    lamh_log = 0.5 * np.log(LAM)
    # log of (sint*gx)^32 * sqrt(LAM) and gy^32 * sqrt(LAM)
    tx = PNORM * (-((x[None, None, :] - sx[..., None]) ** 2)
                  * inv2r2[..., None] + np.log(sint)[..., None]) + lamh_log
    ty = PNORM * (-((y[None, None, :] - sy[..., None]) ** 2)
                  * inv2r2[..., None]) + lamh_log
    gxp = np.where(tx > -87.0, np.exp(tx), 0.0)        # [B,S,W]
    gyp = np.where(ty > -87.0, np.exp(ty), 0.0)        # [B,S,H]

    # device layouts
    gxp_lay = np.ascontiguousarray(
        gxp.transpose(1, 0, 2)).astype(NPB)            # [SR,B,W]
    gyp_lay = np.ascontiguousarray(
        gyp.reshape(B, SR, K, P).transpose(1, 0, 2, 3)).astype(NPB)

    kappa = 1.0 - DARK                                 # [C]
    s1 = -(burn[:, None] * kappa[None, :])             # [B,C]
    s1_lay = np.ascontiguousarray(np.broadcast_to(
        s1.astype(np.float32), (P, B, C)))
    return gxp_lay, gyp_lay, s1_lay


def _edge_weight():
    y = np.linspace(-1.0, 1.0, H)
    x = np.linspace(-1.0, 1.0, W)
    yc, xc = np.meshgrid(y, x, indexing="ij")
    dist = np.sqrt(xc ** 2 + yc ** 2)
    ew = np.exp(2.0 * (dist - 0.7))
    ew = (ew - ew.min()) / (ew.max() - ew.min() + 1e-6)
    # ew_lay[p, k, w] = ew[k*P+p, w]
    return np.ascontiguousarray(
        ew.reshape(K, P, W).transpose(1, 0, 2).astype(np.float16))


_EW = None


def kernel(img, u_xy, u_radius, u_intensity, u_burn, _run_kwargs=None):
    global _EW
    img = np.asarray(img, np.float32)
    # pack to [B, K, P, C*W] bf16: chunk-contiguous DMA blocks
    img_dev = np.ascontiguousarray(
        img.reshape(B, C, K, P, W).transpose(0, 2, 3, 1, 4)
    ).astype(NPB).reshape(B, K, P, C * W)

    gxp_lay, gyp_lay, s1_lay = _host_tables(
        u_xy, u_radius, u_intensity, u_burn)
    if _EW is None:
        _EW = _edge_weight()

    nc = _get_nc()
    core_ids = list(range(NCORES))
    in_maps = []
    for i in core_ids:
        lo, hi = i * BL, (i + 1) * BL
        in_maps.append({
            "img": img_dev[lo:hi],
            "gxp": np.ascontiguousarray(gxp_lay[:, lo:hi]),
            "gyp": np.ascontiguousarray(gyp_lay[:, lo:hi]),
            "ew": _EW,
            "s1": np.ascontiguousarray(s1_lay[:, lo:hi]),
        })
    res = run_bass_kernel_spmd(nc, in_maps, core_ids, **(_run_kwargs or {}))
    out_dev = np.concatenate(
        [np.asarray(res.results[i]["out"]) for i in core_ids], axis=0)
    out = np.ascontiguousarray(
        out_dev.reshape(B, K, P, C, W).transpose(0, 3, 1, 2, 4)
    ).astype(np.float32).reshape(B, C, H, W)
    if _run_kwargs:
        kernel._last_results = res
    return out
